# revision 1
# baseline (speedup 1.0000x reference)
"""Trainium2 Bass kernel for nn_Detect_50431505989817 (YOLO-style detect head).

Computes, for each of 8 images (one per NeuronCore, batch-parallel):
  level0: 1x1 conv (W0 [1548,256]) over x0 [256,64,64] + decode -> [73728, 86]
  level1: 1x1 conv (W1 [1548,512]) over x1 [512,32,32] + decode -> [18432, 86]
  concat -> out [92160, 86]; host stacks cores -> [8, 92160, 86].

Design notes:
  - matmul: stationary = x tile [K=c, 128 hw] in fp16 (same 11-bit mantissa
    as TF32 -> identical rounding error on this data, half the HBM bytes,
    full PE rate), moving = W^T chunk [K=c, n_anchors*86] fp16.
    hw is interleaved: partition p
    covers hw = 512*blk + 4*p + j, with j in [0,4) living in the free dim
    (PSUM bank j).  That makes each partition of the decoded stage tile hold
    4 consecutive output rows => 1376B contiguous DMA runs (full HBM BW;
    <512B runs pay 2x).
  - decode: one ACT Sigmoid per (block, o-chunk) covers xy/conf/cls; wh uses
    exp(t) = sig/(1-sig) on DVE (avoids the 1283ns ACT table swap between the
    Sigmoid and Exp LUT tables); xy adds a host-precomputed grid via fused
    scalar_tensor_tensor; angle is a DVE add reading raw PSUM.
  - host folds anchors/strides/grid into small constant inputs; a nonzero
    conv bias is handled exactly via an appended ones-row/bias-row (K+1).
"""

import math

import numpy as np

import concourse.mybir as mybir
import concourse.tile as tile
from concourse import bacc, bass_utils

F32 = mybir.dt.float32
F16 = mybir.dt.float16
AFT = mybir.ActivationFunctionType
ALU = mybir.AluOpType

NCLS = 80
NA = 18
NCH = 86  # 5 + 1 + NCLS
STRIDES = [8.0, 16.0]
SXY = [1.2, 1.1]
ANCH = [[[10.0, 13.0], [16.0, 30.0], [33.0, 23.0]],
        [[30.0, 61.0], [62.0, 45.0], [59.0, 119.0]]]
ANGLES = [math.pi / 180.0 * a for a in (-60.0, -30.0, 0.0, 30.0, 60.0, 90.0)]

LEVELS = [
    dict(C=256, G=64, HW=4096, s=STRIDES[0], sxy=SXY[0], row0=0),
    dict(C=512, G=32, HW=1024, s=STRIDES[1], sxy=SXY[1], row0=NA * 4096),
]
OUT_ROWS = NA * (4096 + 1024)  # 92160

# o-chunks: (first anchor, n anchors)
OCH = [(0, 5), (5, 5), (10, 5), (15, 3)]

_PROG_CACHE = {}


def _build_program(use_bias: bool):
    nc = bacc.Bacc("TRN2", target_bir_lowering=False, debug=False)

    xs_d, wt_d = [], []
    for li, lv in enumerate(LEVELS):
        K = lv["C"] + (1 if use_bias else 0)
        xs_d.append(nc.dram_tensor(f"xs{li}", [K, lv["HW"]], F16, kind="ExternalInput"))
        wt_d.append(nc.dram_tensor(f"wt{li}", [K, NA * NCH], F16, kind="ExternalInput"))
    # all decode constants packed into one tensor: one DMA, >=512B rows
    # layout: [grid0(64) | grid1(16) | cwh0(36) | cwh1(36) | cang0(18) | cang1(18)]
    cst_d = nc.dram_tensor("cst", [128, 188], F32, kind="ExternalInput")
    out_d = nc.dram_tensor("out", [OUT_ROWS, NCH], F32, kind="ExternalOutput")

    with tile.TileContext(nc) as tc:
        with (
            tc.tile_pool(name="const", bufs=1) as cpool,
            tc.tile_pool(name="stage", bufs=8) as spool,
            tc.tile_pool(name="tmp", bufs=6) as tpool,
            tc.tile_pool(name="psum", bufs=2, space="PSUM") as ppool,
        ):
            zb = cpool.tile([128, 1], F32, tag="zb")
            nc.gpsimd.memset(zb[:], 0.0)

            # resident inputs: packed decode constants first (one small DMA)
            # so the first tile's decode isn't gated on the multi-MB x/W loads
            cst = cpool.tile([128, 188], F32, tag="cst")
            nc.sync.dma_start(cst[:], cst_d.ap()[:])
            grid_t = [cst[:, 0:64], cst[:, 64:80]]
            cwh_t = [cst[:, 80:116], cst[:, 116:152]]
            cang_t = [cst[:, 152:170], cst[:, 170:188]]

            xs_t, wt_t = [], []
            for li, lv in enumerate(LEVELS):
                K = lv["C"] + (1 if use_bias else 0)
                kch = [(k, min(128, K - k)) for k in range(0, K, 128)]
                xts, wts = [], []
                for k0, kc in kch:
                    # fp16 tiles: same 11-bit mantissa as f32r/TF32 (verified
                    # identical decode error on this data) at half the HBM
                    # bytes, full-rate on the PE, and FWL-capable weight loads
                    wt = cpool.tile([kc, NA * NCH], F16, tag=f"wt{li}_{k0}")
                    nc.sync.dma_start(wt[:], wt_d[li].ap()[k0:k0 + kc, :])
                    wts.append(wt)
                    xt = cpool.tile([kc, lv["HW"]], F16, tag=f"xs{li}_{k0}")
                    nc.sync.dma_start(xt[:], xs_d[li].ap()[k0:k0 + kc, :])
                    xts.append(xt)
                xs_t.append(xts)
                wt_t.append(wts)

            for li, lv in enumerate(LEVELS):
                HW, s, sxy, row0 = lv["HW"], lv["s"], lv["sxy"], lv["row0"]
                nb = HW // 512
                nk = len(xs_t[li])
                # [K, HW] viewed as [K, hw//4, j]
                xs_r = [xt.rearrange("k (h j) -> k h j", j=4) for xt in xs_t[li]]
                # DRAM rows of this level as [anchor, block, 128, 344]
                dst_l = out_d.ap()[row0:row0 + NA * HW, :].rearrange(
                    "(a b h j) c -> a b h (j c)", a=NA, b=nb, j=4)

                for b in range(nb):
                    for ci, (a0, na) in enumerate(OCH):
                        P = ppool.tile([128, 2048], F32, tag="psum")
                        for j in range(4):
                            for ki in range(nk):
                                nc.tensor.matmul(
                                    P[:, 512 * j: 512 * j + na * NCH],
                                    xs_r[ki][:, 128 * b: 128 * (b + 1), j],
                                    wt_t[li][ki][:, NCH * a0: NCH * (a0 + na)],
                                    start=(ki == 0), stop=(ki == nk - 1),
                                )

                        S = spool.tile([128, na * 4 * NCH], F32, tag="S")
                        # psum viewed [p, j, a, c] and [p, a, j, c]
                        Pj = P.rearrange("p (j q) -> p j q", q=512)[:, :, 0:na * NCH] \
                            .rearrange("p j (a c) -> p j a c", c=NCH)
                        Pa = Pj.rearrange("p j a c -> p a j c")
                        # stage S layout per partition: [a][j][c]
                        Sa = S.rearrange("p (a j c) -> p a j c", j=4, c=NCH)
                        Sj = Sa.rearrange("p a j c -> p j a c")

                        nc.scalar.activation(Sj, Pj, AFT.Sigmoid, bias=zb[:])

                        # xy: sig*(sxy*s) + grid(hw)
                        gb = grid_t[li][:, 8 * b: 8 * b + 8] \
                            .rearrange("p (a j c) -> p a j c", a=1, c=2) \
                            .broadcast_to([128, na, 4, 2])
                        nc.vector.scalar_tensor_tensor(
                            Sa[:, :, :, 0:2], Sa[:, :, :, 0:2], sxy * s, gb,
                            ALU.mult, ALU.add)

                        # wh: exp(t)*w = w * sig/(1-sig)
                        T = tpool.tile([128, na * 8], F32, tag="T")
                        Tr = T.rearrange("p (a j c) -> p a j c", j=4, c=2)
                        cwb = cwh_t[li][:, 2 * a0: 2 * (a0 + na)] \
                            .rearrange("p (a j c) -> p a j c", j=1, c=2) \
                            .broadcast_to([128, na, 4, 2])
                        nc.vector.tensor_scalar(
                            Tr, Sa[:, :, :, 2:4], -1.0, 1.0, ALU.mult, ALU.add)
                        nc.vector.reciprocal_approx_fast(T[:], T[:])
                        nc.vector.tensor_tensor(Tr, Tr, cwb, ALU.mult)
                        nc.vector.tensor_tensor(
                            Sa[:, :, :, 2:4], Sa[:, :, :, 2:4], Tr, ALU.mult)

                        # angle: t + aa (raw PSUM read)
                        cab = cang_t[li][:, a0:a0 + na] \
                            .rearrange("p (a j c) -> p a j c", j=1, c=1) \
                            .broadcast_to([128, na, 4, 1])
                        nc.vector.tensor_tensor(
                            Sa[:, :, :, 4:5], Pa[:, :, :, 4:5], cab, ALU.add)

                        # store: [p, a, j*c] -> rows (a0+i)*HW + 512b + 4p + j
                        # (partition dim must stay outermost on the SBUF side)
                        dst = dst_l[a0:a0 + na, b, :, :].rearrange("a h q -> h a q")
                        src = S.rearrange("p (a q) -> p a q", q=4 * NCH)
                        nc.sync.dma_start(dst, src)

    nc.compile()
    return nc


def _get_program(use_bias: bool):
    key = bool(use_bias)
    if key not in _PROG_CACHE:
        _PROG_CACHE[key] = _build_program(key)
    return _PROG_CACHE[key]


def _host_consts():
    """Shared (per-core-identical) packed constant input (see cst layout)."""
    grids, cwhs, cangs = [], [], []
    for li, lv in enumerate(LEVELS):
        G, HW, s, sxy = lv["G"], lv["HW"], lv["s"], lv["sxy"]
        nb = HW // 512
        # grid[p, 8b + 2j + c] = value_c(hw = 512b + 4p + j)
        p = np.arange(128)
        b = np.arange(nb)
        j = np.arange(4)
        hw = 512 * b[None, :, None] + 4 * p[:, None, None] + j[None, None, :]
        gx = (hw % G - (sxy - 1.0) / 2.0) * s
        gy = (hw // G - (sxy - 1.0) / 2.0) * s
        grid = np.stack([gx, gy], axis=-1)  # [128, nb, 4, 2]
        grids.append(grid.reshape(128, 8 * nb).astype(np.float32))

        wh = np.array([ANCH[li][a // 6] for a in range(NA)], dtype=np.float32)
        cwhs.append(np.broadcast_to(wh.reshape(1, 2 * NA), (128, 2 * NA)))
        ang = np.array([ANGLES[a % 6] for a in range(NA)], dtype=np.float32)
        cangs.append(np.broadcast_to(ang.reshape(1, NA), (128, NA)))
    cst = np.concatenate(grids + cwhs + cangs, axis=1).astype(np.float32)
    return {"cst": np.ascontiguousarray(cst)}


def kernel(x0, x1, W0, b0, W1, b1):
    x0 = np.ascontiguousarray(x0, dtype=np.float32)
    x1 = np.ascontiguousarray(x1, dtype=np.float32)
    W0 = np.ascontiguousarray(W0, dtype=np.float32)
    W1 = np.ascontiguousarray(W1, dtype=np.float32)
    b0 = np.asarray(b0, dtype=np.float32)
    b1 = np.asarray(b1, dtype=np.float32)
    B = x0.shape[0]
    assert B == 8, f"expected batch 8, got {B}"

    use_bias = bool(np.any(b0) or np.any(b1))
    nc = _get_program(use_bias)

    shared = _host_consts()
    for li, (W, bb) in enumerate(zip((W0, W1), (b0, b1))):
        wt = np.ascontiguousarray(W.T)  # [C, 1548]
        if use_bias:
            wt = np.concatenate([wt, bb.reshape(1, -1)], axis=0)
        shared[f"wt{li}"] = wt.astype(np.float16)

    in_maps = []
    for i in range(B):
        m = dict(shared)
        for li, (x, lv) in enumerate(zip((x0, x1), LEVELS)):
            xs = x[i].reshape(lv["C"], lv["HW"])
            if use_bias:
                xs = np.concatenate(
                    [xs, np.ones((1, lv["HW"]), np.float32)], axis=0)
            m[f"xs{li}"] = np.ascontiguousarray(xs).astype(np.float16)
        in_maps.append(m)

    res = bass_utils.run_bass_kernel_spmd(nc, in_maps, core_ids=list(range(B)))
    return np.stack([res.results[i]["out"] for i in range(B)], axis=0)



# revision 33
# speedup vs baseline: 1.5305x; 1.5305x over previous
"""Trainium2 Bass kernel for nn_Detect_50431505989817 (YOLO-style detect head).

Computes, for each of 8 images (one per NeuronCore, batch-parallel):
  level0: 1x1 conv (W0 [1548,256]) over x0 [256,64,64] + decode -> [73728, 86]
  level1: 1x1 conv (W1 [1548,512]) over x1 [512,32,32] + decode -> [18432, 86]
  concat -> out [92160, 86]; host stacks cores -> [8, 92160, 86].

Design notes:
  - matmul: fp8(e4m3) DoubleRow (2 MACs/cell/cycle): stationary = x tile
    [k2, 2, 128 hw], moving = W^T [k2, 2, n] -- contraction over 2*k2
    channels per pass, half the PE cycles of fp16 and half the x/W HBM
    bytes.  x is pre-scaled *16 and W *256 on host (escapes e4m3
    subnormals; both exact power-of-2), descaled via ACT scale=2^-12.
  - hw interleave: partition p covers hw = 512*blk + 4*p + j, j in [0,4)
    (PSUM bank j); each partition of the decoded stage tile holds 4
    consecutive output rows => 688B contiguous fp16 DMA runs (>=512B
    avoids the 2x small-descriptor penalty).
  - wh precision: the level1 anchors are large (exp amplifies conv error),
    so two fp8 residual matmuls accumulate into the wh PSUM columns:
    r8*v8 (x-quantization residual) and x8*q8 (W-quantization residual),
    bringing the wh error back to ~fp16 level at fp8 speed.
  - decode: one ACT Sigmoid per (block, o-chunk) covers xy/conf/cls; wh
    uses exp(t) = sig/(1-sig) on DVE (avoids the 1283ns ACT table swap);
    xy adds a host-precomputed grid via fused scalar_tensor_tensor; angle
    reads raw PSUM via scalar_tensor_tensor (descale + anchor-angle add).
  - fp16 output store (host upcasts to f32): halves the dominant HBM
    store traffic; fp16 rel err ~5e-4 vs the 2e-2 scale-rel gate.
  - host folds anchors/strides/grid into one packed constant tensor.
"""

import math

import numpy as np
import ml_dtypes

import concourse.mybir as mybir
import concourse.tile as tile
from concourse import bacc, bass_utils

F32 = mybir.dt.float32
F16 = mybir.dt.float16
F8 = mybir.dt.float8e4
AFT = mybir.ActivationFunctionType
ALU = mybir.AluOpType
DROW = mybir.MatmulPerfMode.DoubleRow

E4M3 = ml_dtypes.float8_e4m3  # TRN float8e4 (IEEE-ish, max +-240)

NCLS = 80
NA = 18
NCH = 86  # 5 + 1 + NCLS
STRIDES = [8.0, 16.0]
SXY = [1.2, 1.1]
ANCH = [[[10.0, 13.0], [16.0, 30.0], [33.0, 23.0]],
        [[30.0, 61.0], [62.0, 45.0], [59.0, 119.0]]]
ANGLES = [math.pi / 180.0 * a for a in (-60.0, -30.0, 0.0, 30.0, 60.0, 90.0)]

LEVELS = [
    dict(C=256, G=64, HW=4096, s=STRIDES[0], sxy=SXY[0], row0=0),
    dict(C=512, G=32, HW=1024, s=STRIDES[1], sxy=SXY[1], row0=NA * 4096),
]
OUT_ROWS = NA * (4096 + 1024)  # 92160

SC_X = 16.0     # host pre-scale on x (exact power of 2)
SC_W = 256.0    # host pre-scale on W (keeps w*SC_W in e4m3 normal range)
DESC = 1.0 / (SC_X * SC_W)  # 2^-12, applied by ACT / angle ops

# o-chunks: (first anchor, n anchors); na*86 <= 512 = one PSUM bank per j
OCH = [(0, 5), (5, 5), (10, 5), (15, 3)]

# block schedule: interleave level1 blocks among level0 blocks so the
# input DMA stream and decode work stay evenly paced
SCHED = [(0, 0), (0, 1), (0, 2), (0, 3), (1, 0),
         (0, 4), (0, 5), (0, 6), (1, 1), (0, 7)]

_PROG_CACHE = {}


def _build_program(use_bias: bool):
    nc = bacc.Bacc("TRN2", target_bir_lowering=False, debug=False)

    # K channels (+2 bias rows when used, so the channel count stays even
    # for DoubleRow pairing: rows [ones, zeros] x weight rows [b, 0])
    Ks = [lv["C"] + (2 if use_bias else 0) for lv in LEVELS]
    k2s = [K // 2 for K in Ks]
    nks = [(k2 + 127) // 128 for k2 in k2s]

    # ISA dual-fp8 LDWEIGHTS/matmul restriction: every non-innermost free-AP
    # step must be 16B-aligned.  x8 uses block-local planes
    # (col = 1024*b + 512*i + 4*p + j -> i step 512), W8 pads each i-plane
    # to 2048 cols, wres to 80.
    WPAD = 2048
    RPAD = 80
    xs_d, wt_d = [], []
    for li, lv in enumerate(LEVELS):
        xs_d.append(nc.dram_tensor(f"xs{li}", [k2s[li], 2 * lv["HW"]], F8,
                                   kind="ExternalInput"))
        wt_d.append(nc.dram_tensor(f"wt{li}", [k2s[li], 2 * WPAD], F8,
                                   kind="ExternalInput"))
    # level1 wh residual operands: r8 (x residual) and packed correction
    # weights wres = [v8(36) | q8(36) | pad] per i-plane
    r8_d = nc.dram_tensor("r8", [k2s[1], 2 * LEVELS[1]["HW"]], F8,
                          kind="ExternalInput")
    wres_d = nc.dram_tensor("wres", [k2s[1], 2 * RPAD], F8,
                            kind="ExternalInput")
    # packed decode constants:
    # layout: [grid0(64) | grid1(16) | cwh0(36) | cwh1(36) | cang0(18) | cang1(18)]
    cst_d = nc.dram_tensor("cst", [128, 188], F32, kind="ExternalInput")
    # fp16 output (host upcasts)
    out_d = nc.dram_tensor("out", [OUT_ROWS, NCH], F16, kind="ExternalOutput")

    with tile.TileContext(nc) as tc:
        with (
            tc.tile_pool(name="const", bufs=1) as cpool,
            tc.tile_pool(name="stage", bufs=8) as spool,
            tc.tile_pool(name="tmp", bufs=6) as tpool,
            tc.tile_pool(name="psum", bufs=2, space="PSUM") as ppool,
        ):
            zb = cpool.tile([128, 1], F32, tag="zb")
            nc.gpsimd.memset(zb[:], 0.0)
            # warm the Sigmoid ACT table during the DMA lead-in (the
            # implicit 1283ns table load would otherwise delay the first
            # real activation)
            warm = cpool.tile([128, 1], F32, tag="warm")
            nc.scalar.activation(warm[:], zb[:], AFT.Sigmoid, bias=zb[:])

            cst = cpool.tile([128, 188], F32, tag="cst")
            grid_t = [cst[:, 0:64], cst[:, 64:80]]
            cwh_t = [cst[:, 80:116], cst[:, 116:152]]
            cang_t = [cst[:, 152:170], cst[:, 170:188]]

            # resident input tiles (alloc now, fill in pipeline order)
            xs_t, wt_t = [], []
            kch_l = []
            for li in range(2):
                kch = [(k, min(128, k2s[li] - k)) for k in range(0, k2s[li], 128)]
                kch_l.append(kch)
                xts, wts = [], []
                for k0, kc in kch:
                    wts.append(cpool.tile([kc, 2 * WPAD], F8,
                                          name=f"wt{li}_{k0}", tag=f"wt{li}_{k0}"))
                    xts.append(cpool.tile([kc, 2 * LEVELS[li]["HW"]], F8,
                                          name=f"xs{li}_{k0}", tag=f"xs{li}_{k0}"))
                xs_t.append(xts)
                wt_t.append(wts)
            r8_t = [cpool.tile([kc, 2 * LEVELS[1]["HW"]], F8, name=f"r8_{k0}",
                               tag=f"r8_{k0}") for k0, kc in kch_l[1]]
            wres_t = [cpool.tile([kc, 2 * RPAD], F8, name=f"wres_{k0}",
                                 tag=f"wres_{k0}") for k0, kc in kch_l[1]]

            HW1 = LEVELS[1]["HW"]

            # x8 packing has i INNERMOST (col = 2*hw + i): a block's piece is
            # one contiguous 1024-col range, so the per-block loads don't
            # bounding-box-overlap later pieces in the hazard tracker
            def load_x_piece(li, b):
                for ki, (k0, kc) in enumerate(kch_l[li]):
                    nc.sync.dma_start(
                        xs_t[li][ki][:, 1024 * b: 1024 * (b + 1)],
                        xs_d[li].ap()[k0:k0 + kc, 1024 * b: 1024 * (b + 1)])

            def load_x_rest(li, b0):
                for ki, (k0, kc) in enumerate(kch_l[li]):
                    nc.sync.dma_start(xs_t[li][ki][:, 1024 * b0:],
                                      xs_d[li].ap()[k0:k0 + kc, 1024 * b0:])

            # load order: first block's deps first => first matmul ~2.5us.
            # W planes loaded without the padding cols
            def load_w(li):
                for ki, (k0, kc) in enumerate(kch_l[li]):
                    for i in range(2):
                        nc.sync.dma_start(
                            wt_t[li][ki][:, WPAD * i: WPAD * i + NA * NCH],
                            wt_d[li].ap()[k0:k0 + kc,
                                          WPAD * i: WPAD * i + NA * NCH])

            load_w(0)
            load_x_piece(0, 0)
            nc.sync.dma_start(cst[:], cst_d.ap()[:])
            load_x_piece(0, 1)
            load_w(1)
            for ki, (k0, kc) in enumerate(kch_l[1]):
                nc.sync.dma_start(wres_t[ki][:], wres_d.ap()[k0:k0 + kc, :])
            load_x_rest(0, 2)
            for ki, (k0, kc) in enumerate(kch_l[1]):
                nc.sync.dma_start(xs_t[1][ki][:], xs_d[1].ap()[k0:k0 + kc, :])
            for ki, (k0, kc) in enumerate(kch_l[1]):
                nc.sync.dma_start(r8_t[ki][:], r8_d.ap()[k0:k0 + kc, :])

            # x8 col = 1024*b + 512*i + 4*p + j -> view [k2, b, i, p, j];
            # per (b, j) the lhsT slice is [k2, i(step 512), p(step 4)]
            def xview(t):
                return t.rearrange("k (b i p j) -> k b i p j", i=2, p=128, j=4)

            xs_r = [[xview(t) for t in xs_t[li]] for li in range(2)]
            r8_r = [xview(t) for t in r8_t]
            wt_r = [[t.rearrange("k (i n) -> k i n", i=2) for t in wt_t[li]]
                    for li in range(2)]
            wres_r = [t.rearrange("k (i n) -> k i n", i=2) for t in wres_t]

            dst_ll = [out_d.ap()[lv["row0"]:lv["row0"] + NA * lv["HW"], :]
                      .rearrange("(a b h j) c -> a b h (j c)",
                                 a=NA, b=lv["HW"] // 512, j=4)
                      for lv in LEVELS]

            for li, b in SCHED:
                lv = LEVELS[li]
                s, sxy = lv["s"], lv["sxy"]
                nk = nks[li]

                for ci, (a0, na) in enumerate(OCH):
                    P = ppool.tile([128, 2048], F32, tag="psum")
                    for j in range(4):
                        for ki in range(nk):
                            nc.tensor.matmul(
                                P[:, 512 * j: 512 * j + na * NCH],
                                xs_r[li][ki][:, b, :, :, j],
                                wt_r[li][ki][:, :, NCH * a0: NCH * (a0 + na)],
                                start=(ki == 0), stop=(ki == nk - 1),
                                perf_mode=DROW,
                            )
                    # psum viewed [p, j, a, c] and [p, a, j, c]
                    Pj = P.rearrange("p (j q) -> p j q", q=512)[:, :, 0:na * NCH] \
                        .rearrange("p j (a c) -> p j a c", c=NCH)
                    Pa = Pj.rearrange("p j a c -> p a j c")

                    if li == 1:
                        # wh residual corrections accumulate into the wh
                        # columns (c=2,3 per anchor): r8*v8 + x8*q8
                        for j in range(4):
                            for ki in range(nk):
                                for lhs, n0 in ((r8_r[ki], 0),
                                                (xs_r[1][ki], 2 * NA)):
                                    nc.tensor.matmul(
                                        Pj[:, j, :, 2:4],
                                        lhs[:, b, :, :, j],
                                        wres_r[ki][:, :, n0 + 2 * a0:
                                                   n0 + 2 * (a0 + na)],
                                        start=False, stop=False,
                                        perf_mode=DROW,
                                        skip_group_check=True,
                                    )

                    S = spool.tile([128, na * 4 * NCH], F16, tag="S")
                    # stage S layout per partition: [a][j][c]
                    Sa = S.rearrange("p (a j c) -> p a j c", j=4, c=NCH)
                    Sj = Sa.rearrange("p a j c -> p j a c")

                    nc.scalar.activation(Sj, Pj, AFT.Sigmoid, bias=zb[:],
                                         scale=DESC)

                    # angle (the only other PSUM reader) right after ACT:
                    # PSUM frees at ACT-end, next chunk's matmuls start early.
                    # 3D APs (p, a, j) — TensorScalarPtr rejects 4D patterns
                    cab = cang_t[li][:, a0:a0 + na] \
                        .rearrange("p (a j) -> p a j", j=1) \
                        .broadcast_to([128, na, 4])
                    nc.vector.scalar_tensor_tensor(
                        Sa[:, :, :, 4:5].rearrange("p a j c -> p a (j c)"),
                        Pa[:, :, :, 4:5].rearrange("p a j c -> p a (j c)"),
                        DESC, cab, ALU.mult, ALU.add)

                    # xy: sig*(sxy*s) + grid(hw)
                    gb = grid_t[li][:, 8 * b: 8 * b + 8] \
                        .rearrange("p (a j c) -> p a j c", a=1, c=2) \
                        .broadcast_to([128, na, 4, 2])
                    nc.vector.scalar_tensor_tensor(
                        Sa[:, :, :, 0:2], Sa[:, :, :, 0:2], sxy * s, gb,
                        ALU.mult, ALU.add)

                    # wh: exp(t)*w = w * sig/(1-sig)  (T kept f32)
                    T = tpool.tile([128, na * 8], F32, tag="T")
                    Tr = T.rearrange("p (a j c) -> p a j c", j=4, c=2)
                    cwb = cwh_t[li][:, 2 * a0: 2 * (a0 + na)] \
                        .rearrange("p (a j c) -> p a j c", j=1, c=2) \
                        .broadcast_to([128, na, 4, 2])
                    nc.vector.tensor_scalar(
                        Tr, Sa[:, :, :, 2:4], -1.0, 1.0, ALU.mult, ALU.add)
                    nc.vector.reciprocal_approx_fast(T[:], T[:])
                    nc.vector.tensor_tensor(Tr, Tr, cwb, ALU.mult)
                    nc.vector.tensor_tensor(
                        Sa[:, :, :, 2:4], Sa[:, :, :, 2:4], Tr, ALU.mult)

                    # store: [p, a, j*c] -> rows (a0+i)*HW + 512b + 4p + j
                    dst = dst_ll[li][a0:a0 + na, b, :, :].rearrange(
                        "a h q -> h a q")
                    src = S.rearrange("p (a q) -> p a q", q=4 * NCH)
                    nc.sync.dma_start(dst, src)

    nc.compile()
    return nc


def _get_program(use_bias: bool):
    key = bool(use_bias)
    if key not in _PROG_CACHE:
        _PROG_CACHE[key] = _build_program(key)
    return _PROG_CACHE[key]


def _host_consts():
    """Shared (per-core-identical) packed constant input (see cst layout)."""
    grids, cwhs, cangs = [], [], []
    for li, lv in enumerate(LEVELS):
        G, HW, s, sxy = lv["G"], lv["HW"], lv["s"], lv["sxy"]
        nb = HW // 512
        # grid[p, 8b + 2j + c] = value_c(hw = 512b + 4p + j)
        p = np.arange(128)
        b = np.arange(nb)
        j = np.arange(4)
        hw = 512 * b[None, :, None] + 4 * p[:, None, None] + j[None, None, :]
        gx = (hw % G - (sxy - 1.0) / 2.0) * s
        gy = (hw // G - (sxy - 1.0) / 2.0) * s
        grid = np.stack([gx, gy], axis=-1)  # [128, nb, 4, 2]
        grids.append(grid.reshape(128, 8 * nb).astype(np.float32))

        wh = np.array([ANCH[li][a // 6] for a in range(NA)], dtype=np.float32)
        cwhs.append(np.broadcast_to(wh.reshape(1, 2 * NA), (128, 2 * NA)))
        ang = np.array([ANGLES[a % 6] for a in range(NA)], dtype=np.float32)
        cangs.append(np.broadcast_to(ang.reshape(1, NA), (128, NA)))
    cst = np.concatenate(grids + cwhs + cangs, axis=1).astype(np.float32)
    return {"cst": np.ascontiguousarray(cst)}


def _pack_x(a):
    """[K, HW] -> [K/2, 2*HW], col = 1024*b + 512*i + (hw % 512)."""
    K, N = a.shape
    nb = N // 512
    v = a.reshape(K // 2, 2, nb, 512).transpose(0, 2, 1, 3)
    return np.ascontiguousarray(v.reshape(K // 2, 2 * N))


def _pack_w(a, pad):
    """[K, N] -> [K/2, 2*pad], col = i*pad + n (planes padded to 16B mult)."""
    K, N = a.shape
    v = np.zeros((K // 2, 2, pad), dtype=a.dtype)
    v[:, :, :N] = a.reshape(K // 2, 2, N)
    return np.ascontiguousarray(v.reshape(K // 2, 2 * pad))


def kernel(x0, x1, W0, b0, W1, b1):
    x0 = np.asarray(x0, dtype=np.float32)
    x1 = np.asarray(x1, dtype=np.float32)
    W0 = np.asarray(W0, dtype=np.float32)
    W1 = np.asarray(W1, dtype=np.float32)
    b0 = np.asarray(b0, dtype=np.float32)
    b1 = np.asarray(b1, dtype=np.float32)
    B = x0.shape[0]
    assert B == 8, f"expected batch 8, got {B}"

    use_bias = bool(np.any(b0) or np.any(b1))
    nc = _get_program(use_bias)

    shared = _host_consts()
    whcols = np.array([86 * a + 2 + c for a in range(NA) for c in range(2)])
    for li, (W, bb) in enumerate(zip((W0, W1), (b0, b1))):
        wt = np.ascontiguousarray(W.T)  # [C, 1548]
        if use_bias:
            wt = np.concatenate(
                [wt, bb.reshape(1, -1), np.zeros_like(bb).reshape(1, -1)],
                axis=0)
        w8 = (wt * SC_W).astype(E4M3)
        shared[f"wt{li}"] = _pack_w(w8, 2048)
        if li == 1:
            # wh correction weights: v8 ~ w*16 (for the x-residual term),
            # q8 = W-quantization residual (256w - w8)
            v8 = (wt[:, whcols] * SC_X).astype(E4M3)
            q8 = (wt[:, whcols] * SC_W
                  - w8[:, whcols].astype(np.float32)).astype(E4M3)
            wres = np.concatenate(
                [v8.astype(np.float32), q8.astype(np.float32),
                 np.zeros((wt.shape[0], 8), np.float32)],
                axis=1).astype(E4M3)
            shared["wres"] = _pack_w(wres, 80)

    in_maps = []
    for i in range(B):
        m = dict(shared)
        for li, (x, lv) in enumerate(zip((x0, x1), LEVELS)):
            xs = x[i].reshape(lv["C"], lv["HW"]) * SC_X
            if use_bias:
                xs = np.concatenate(
                    [xs, np.full((1, lv["HW"]), SC_X, np.float32),
                     np.zeros((1, lv["HW"]), np.float32)], axis=0)
            x8 = xs.astype(E4M3)
            m[f"xs{li}"] = _pack_x(x8)
            if li == 1:
                r8 = ((xs - x8.astype(np.float32)) * SC_X).astype(E4M3)
                m["r8"] = _pack_x(r8)
        in_maps.append(m)

    res = bass_utils.run_bass_kernel_spmd(nc, in_maps, core_ids=list(range(B)))
    out = np.stack([res.results[i]["out"] for i in range(B)], axis=0)
    return np.ascontiguousarray(out.astype(np.float32))


# revision 47
# speedup vs baseline: 1.5418x; 1.0074x over previous
"""Trainium2 Bass kernel for nn_Detect_50431505989817 (YOLO-style detect head).

Computes, for each of 8 images (one per NeuronCore, batch-parallel):
  level0: 1x1 conv (W0 [1548,256]) over x0 [256,64,64] + decode -> [73728, 86]
  level1: 1x1 conv (W1 [1548,512]) over x1 [512,32,32] + decode -> [18432, 86]
  concat -> out [92160, 86]; host stacks cores -> [8, 92160, 86].

Design notes:
  - matmul: fp8(e4m3) DoubleRow (2 MACs/cell/cycle): stationary = x tile
    [k2, 2, 128 hw], moving = W^T [k2, 2, n] -- contraction over 2*k2
    channels per pass, half the PE cycles of fp16 and half the x/W HBM
    bytes.  x is pre-scaled *16 and W *256 on host (escapes e4m3
    subnormals; both exact power-of-2), descaled via ACT scale=2^-12.
  - hw interleave: partition p covers hw = 512*blk + 4*p + j, j in [0,4)
    (PSUM bank j); each partition of the decoded stage tile holds 4
    consecutive output rows => 688B contiguous fp16 DMA runs (>=512B
    avoids the 2x small-descriptor penalty).
  - wh precision: the level1 anchors are large (exp amplifies conv error),
    so two fp8 residual matmuls accumulate into the wh PSUM columns:
    r8*v8 (x-quantization residual) and x8*q8 (W-quantization residual),
    bringing the wh error back to ~fp16 level at fp8 speed.
  - decode: one ACT Sigmoid per (block, o-chunk) covers xy/conf/cls; wh
    uses exp(t) = sig/(1-sig) on DVE (avoids the 1283ns ACT table swap);
    xy adds a host-precomputed grid via fused scalar_tensor_tensor; angle
    reads raw PSUM via scalar_tensor_tensor (descale + anchor-angle add).
  - fp16 output store (host upcasts to f32): halves the dominant HBM
    store traffic; fp16 rel err ~5e-4 vs the 2e-2 scale-rel gate.
  - host folds anchors/strides/grid into one packed constant tensor.
"""

import math

import numpy as np
import ml_dtypes

import concourse.mybir as mybir
import concourse.tile as tile
from concourse import bacc, bass_utils

F32 = mybir.dt.float32
F16 = mybir.dt.float16
F8 = mybir.dt.float8e4
AFT = mybir.ActivationFunctionType
ALU = mybir.AluOpType
DROW = mybir.MatmulPerfMode.DoubleRow

E4M3 = ml_dtypes.float8_e4m3  # TRN float8e4 (IEEE-ish, max +-240)

NCLS = 80
NA = 18
NCH = 86  # 5 + 1 + NCLS
STRIDES = [8.0, 16.0]
SXY = [1.2, 1.1]
ANCH = [[[10.0, 13.0], [16.0, 30.0], [33.0, 23.0]],
        [[30.0, 61.0], [62.0, 45.0], [59.0, 119.0]]]
ANGLES = [math.pi / 180.0 * a for a in (-60.0, -30.0, 0.0, 30.0, 60.0, 90.0)]

LEVELS = [
    dict(C=256, G=64, HW=4096, s=STRIDES[0], sxy=SXY[0], row0=0),
    dict(C=512, G=32, HW=1024, s=STRIDES[1], sxy=SXY[1], row0=NA * 4096),
]
OUT_ROWS = NA * (4096 + 1024)  # 92160

SC_X = 16.0     # host pre-scale on x (exact power of 2)
SC_W = 256.0    # host pre-scale on W (keeps w*SC_W in e4m3 normal range)
DESC = 1.0 / (SC_X * SC_W)  # 2^-12, applied by ACT / angle ops

# o-chunks: (first anchor, n anchors); na*86 <= 512 = one PSUM bank per j
OCH = [(0, 5), (5, 5), (10, 5), (15, 3)]

# block schedule: interleave level1 blocks among level0 blocks so the
# input DMA stream and decode work stay evenly paced
SCHED = [(0, 0), (0, 1), (0, 2), (0, 3), (1, 0),
         (0, 4), (0, 5), (0, 6), (1, 1), (0, 7)]

_PROG_CACHE = {}


def _build_program(use_bias: bool):
    nc = bacc.Bacc("TRN2", target_bir_lowering=False, debug=False)

    # K channels (+2 bias rows when used, so the channel count stays even
    # for DoubleRow pairing: rows [ones, zeros] x weight rows [b, 0])
    Ks = [lv["C"] + (2 if use_bias else 0) for lv in LEVELS]
    k2s = [K // 2 for K in Ks]
    nks = [(k2 + 127) // 128 for k2 in k2s]

    # ISA dual-fp8 LDWEIGHTS/matmul restriction: every non-innermost free-AP
    # step must be 16B-aligned.  x8 uses block-local planes
    # (col = 1024*b + 512*i + 4*p + j -> i step 512), W8 pads each i-plane
    # to 2048 cols, wres to 80.
    WPAD = 2048
    RPAD = 80
    xs_d, wt_d = [], []
    for li, lv in enumerate(LEVELS):
        xs_d.append(nc.dram_tensor(f"xs{li}", [k2s[li], 2 * lv["HW"]], F8,
                                   kind="ExternalInput"))
        wt_d.append(nc.dram_tensor(f"wt{li}", [k2s[li], 2 * WPAD], F8,
                                   kind="ExternalInput"))
    # level1 wh residual operands: r8 (x residual) and packed correction
    # weights wres = [v8(36) | q8(36) | pad] per i-plane
    r8_d = nc.dram_tensor("r8", [k2s[1], 2 * LEVELS[1]["HW"]], F8,
                          kind="ExternalInput")
    wres_d = nc.dram_tensor("wres", [k2s[1], 2 * RPAD], F8,
                            kind="ExternalInput")
    # packed decode constants:
    # layout: [grid0(64) | grid1(16) | cwh0(36) | cwh1(36) | cang0(18) | cang1(18)]
    cst_d = nc.dram_tensor("cst", [128, 188], F32, kind="ExternalInput")
    # fp16 output (host upcasts)
    out_d = nc.dram_tensor("out", [OUT_ROWS, NCH], F16, kind="ExternalOutput")

    with tile.TileContext(nc) as tc:
        with (
            tc.tile_pool(name="const", bufs=1) as cpool,
            tc.tile_pool(name="stage", bufs=8) as spool,
            tc.tile_pool(name="tmp", bufs=6) as tpool,
            tc.tile_pool(name="psum", bufs=2, space="PSUM") as ppool,
        ):
            zb = cpool.tile([128, 1], F32, tag="zb")
            nc.gpsimd.memset(zb[:], 0.0)
            # warm the Sigmoid ACT table during the DMA lead-in (the
            # implicit 1283ns table load would otherwise delay the first
            # real activation)
            warm = cpool.tile([128, 1], F32, tag="warm")
            nc.scalar.activation(warm[:], zb[:], AFT.Sigmoid, bias=zb[:])

            cst = cpool.tile([128, 188], F32, tag="cst")
            grid_t = [cst[:, 0:64], cst[:, 64:80]]
            cwh_t = [cst[:, 80:116], cst[:, 116:152]]
            cang_t = [cst[:, 152:170], cst[:, 170:188]]

            # resident input tiles (alloc now, fill in pipeline order)
            xs_t, wt_t = [], []
            kch_l = []
            for li in range(2):
                kch = [(k, min(128, k2s[li] - k)) for k in range(0, k2s[li], 128)]
                kch_l.append(kch)
                xts, wts = [], []
                for k0, kc in kch:
                    wts.append(cpool.tile([kc, 2 * WPAD], F8,
                                          name=f"wt{li}_{k0}", tag=f"wt{li}_{k0}"))
                    xts.append(cpool.tile([kc, 2 * LEVELS[li]["HW"]], F8,
                                          name=f"xs{li}_{k0}", tag=f"xs{li}_{k0}"))
                xs_t.append(xts)
                wt_t.append(wts)
            r8_t = [cpool.tile([kc, 2 * LEVELS[1]["HW"]], F8, name=f"r8_{k0}",
                               tag=f"r8_{k0}") for k0, kc in kch_l[1]]
            wres_t = [cpool.tile([kc, 2 * RPAD], F8, name=f"wres_{k0}",
                                 tag=f"wres_{k0}") for k0, kc in kch_l[1]]

            HW1 = LEVELS[1]["HW"]

            # x8 packing has i INNERMOST (col = 2*hw + i): a block's piece is
            # one contiguous 1024-col range, so the per-block loads don't
            # bounding-box-overlap later pieces in the hazard tracker
            def load_x_piece(li, b):
                for ki, (k0, kc) in enumerate(kch_l[li]):
                    nc.sync.dma_start(
                        xs_t[li][ki][:, 1024 * b: 1024 * (b + 1)],
                        xs_d[li].ap()[k0:k0 + kc, 1024 * b: 1024 * (b + 1)])

            def load_x_rest(li, b0):
                for ki, (k0, kc) in enumerate(kch_l[li]):
                    nc.sync.dma_start(xs_t[li][ki][:, 1024 * b0:],
                                      xs_d[li].ap()[k0:k0 + kc, 1024 * b0:])

            # load order: first block's deps first => first matmul ~2.5us.
            # W planes loaded without the padding cols
            def load_w(li):
                for ki, (k0, kc) in enumerate(kch_l[li]):
                    for i in range(2):
                        nc.sync.dma_start(
                            wt_t[li][ki][:, WPAD * i: WPAD * i + NA * NCH],
                            wt_d[li].ap()[k0:k0 + kc,
                                          WPAD * i: WPAD * i + NA * NCH])

            load_w(0)
            load_x_piece(0, 0)
            nc.sync.dma_start(cst[:], cst_d.ap()[:])
            load_x_piece(0, 1)
            load_w(1)
            for ki, (k0, kc) in enumerate(kch_l[1]):
                nc.sync.dma_start(wres_t[ki][:], wres_d.ap()[k0:k0 + kc, :])
            load_x_rest(0, 2)
            for ki, (k0, kc) in enumerate(kch_l[1]):
                nc.sync.dma_start(xs_t[1][ki][:], xs_d[1].ap()[k0:k0 + kc, :])
            for ki, (k0, kc) in enumerate(kch_l[1]):
                nc.sync.dma_start(r8_t[ki][:], r8_d.ap()[k0:k0 + kc, :])

            # x8 col = 1024*b + 512*i + 4*p + j -> view [k2, b, i, p, j];
            # per (b, j) the lhsT slice is [k2, i(step 512), p(step 4)]
            def xview(t):
                return t.rearrange("k (b i p j) -> k b i p j", i=2, p=128, j=4)

            xs_r = [[xview(t) for t in xs_t[li]] for li in range(2)]
            r8_r = [xview(t) for t in r8_t]
            wt_r = [[t.rearrange("k (i n) -> k i n", i=2) for t in wt_t[li]]
                    for li in range(2)]
            wres_r = [t.rearrange("k (i n) -> k i n", i=2) for t in wres_t]

            dst_ll = [out_d.ap()[lv["row0"]:lv["row0"] + NA * lv["HW"], :]
                      .rearrange("(a b h j) c -> a b h (j c)",
                                 a=NA, b=lv["HW"] // 512, j=4)
                      for lv in LEVELS]

            for si, (li, b) in enumerate(SCHED):
                lv = LEVELS[li]
                s, sxy = lv["s"], lv["sxy"]
                nk = nks[li]

                # na=3 chunk FIRST: its short matmul chain hides inside the
                # previous block's last (na=5) ACT at the PSUM ring handoff.
                # Final block keeps na=3 LAST for a small tail store.
                och = OCH if si == len(SCHED) - 1 else [OCH[3]] + OCH[0:3]
                for ci, (a0, na) in enumerate(och):
                    P = ppool.tile([128, 2048], F32, tag="psum")
                    for j in range(4):
                        for ki in range(nk):
                            nc.tensor.matmul(
                                P[:, 512 * j: 512 * j + na * NCH],
                                xs_r[li][ki][:, b, :, :, j],
                                wt_r[li][ki][:, :, NCH * a0: NCH * (a0 + na)],
                                start=(ki == 0), stop=(ki == nk - 1),
                                perf_mode=DROW,
                            )
                    # psum viewed [p, j, a, c] and [p, a, j, c]
                    Pj = P.rearrange("p (j q) -> p j q", q=512)[:, :, 0:na * NCH] \
                        .rearrange("p j (a c) -> p j a c", c=NCH)
                    Pa = Pj.rearrange("p j a c -> p a j c")

                    if li == 1:
                        # wh residual corrections accumulate into the wh
                        # columns (c=2,3 per anchor): r8*v8 + x8*q8
                        for j in range(4):
                            for ki in range(nk):
                                for lhs, n0 in ((r8_r[ki], 0),
                                                (xs_r[1][ki], 2 * NA)):
                                    nc.tensor.matmul(
                                        Pj[:, j, :, 2:4],
                                        lhs[:, b, :, :, j],
                                        wres_r[ki][:, :, n0 + 2 * a0:
                                                   n0 + 2 * (a0 + na)],
                                        start=False, stop=False,
                                        perf_mode=DROW,
                                        skip_group_check=True,
                                    )

                    S = spool.tile([128, na * 4 * NCH], F16, tag="S")
                    # stage S layout per partition: [a][j][c]
                    Sa = S.rearrange("p (a j c) -> p a j c", j=4, c=NCH)
                    Sj = Sa.rearrange("p a j c -> p j a c")

                    nc.scalar.activation(Sj, Pj, AFT.Sigmoid, bias=zb[:],
                                         scale=DESC)

                    # angle (the only other PSUM reader) right after ACT:
                    # PSUM frees soon after ACT-end, so the next chunks'
                    # matmuls start early.
                    # 3D APs (p, a, j) — TensorScalarPtr rejects 4D patterns
                    cab = cang_t[li][:, a0:a0 + na] \
                        .rearrange("p (a j) -> p a j", j=1) \
                        .broadcast_to([128, na, 4])
                    nc.vector.scalar_tensor_tensor(
                        Sa[:, :, :, 4:5].rearrange("p a j c -> p a (j c)"),
                        Pa[:, :, :, 4:5].rearrange("p a j c -> p a (j c)"),
                        DESC, cab, ALU.mult, ALU.add)

                    # xy: sig*(sxy*s) + grid(hw)
                    gb = grid_t[li][:, 8 * b: 8 * b + 8] \
                        .rearrange("p (a j c) -> p a j c", a=1, c=2) \
                        .broadcast_to([128, na, 4, 2])
                    nc.vector.scalar_tensor_tensor(
                        Sa[:, :, :, 0:2], Sa[:, :, :, 0:2], sxy * s, gb,
                        ALU.mult, ALU.add)

                    # wh: exp(t)*w = w * sig/(1-sig)  (T kept f32)
                    T = tpool.tile([128, na * 8], F32, name="T", tag="T")
                    Tr = T.rearrange("p (a j c) -> p a j c", j=4, c=2)
                    cwb = cwh_t[li][:, 2 * a0: 2 * (a0 + na)] \
                        .rearrange("p (a j c) -> p a j c", j=1, c=2) \
                        .broadcast_to([128, na, 4, 2])
                    nc.vector.tensor_scalar(
                        Tr, Sa[:, :, :, 2:4], -1.0, 1.0, ALU.mult, ALU.add)
                    nc.vector.reciprocal_approx_fast(T[:], T[:])
                    nc.vector.tensor_tensor(Tr, Tr, cwb, ALU.mult)
                    nc.vector.tensor_tensor(
                        Sa[:, :, :, 2:4], Sa[:, :, :, 2:4], Tr, ALU.mult)

                    # store: [p, a, j*c] -> rows (a0+i)*HW + 512b + 4p + j
                    dst = dst_ll[li][a0:a0 + na, b, :, :].rearrange(
                        "a h q -> h a q")
                    src_ap = S.rearrange("p (a q) -> p a q", q=4 * NCH)
                    nc.sync.dma_start(dst, src_ap)

    nc.compile()
    return nc


def _get_program(use_bias: bool):
    key = bool(use_bias)
    if key not in _PROG_CACHE:
        _PROG_CACHE[key] = _build_program(key)
    return _PROG_CACHE[key]


def _host_consts():
    """Shared (per-core-identical) packed constant input (see cst layout)."""
    grids, cwhs, cangs = [], [], []
    for li, lv in enumerate(LEVELS):
        G, HW, s, sxy = lv["G"], lv["HW"], lv["s"], lv["sxy"]
        nb = HW // 512
        # grid[p, 8b + 2j + c] = value_c(hw = 512b + 4p + j)
        p = np.arange(128)
        b = np.arange(nb)
        j = np.arange(4)
        hw = 512 * b[None, :, None] + 4 * p[:, None, None] + j[None, None, :]
        gx = (hw % G - (sxy - 1.0) / 2.0) * s
        gy = (hw // G - (sxy - 1.0) / 2.0) * s
        grid = np.stack([gx, gy], axis=-1)  # [128, nb, 4, 2]
        grids.append(grid.reshape(128, 8 * nb).astype(np.float32))

        wh = np.array([ANCH[li][a // 6] for a in range(NA)], dtype=np.float32)
        cwhs.append(np.broadcast_to(wh.reshape(1, 2 * NA), (128, 2 * NA)))
        ang = np.array([ANGLES[a % 6] for a in range(NA)], dtype=np.float32)
        cangs.append(np.broadcast_to(ang.reshape(1, NA), (128, NA)))
    cst = np.concatenate(grids + cwhs + cangs, axis=1).astype(np.float32)
    return {"cst": np.ascontiguousarray(cst)}


def _pack_x(a):
    """[K, HW] -> [K/2, 2*HW], col = 1024*b + 512*i + (hw % 512)."""
    K, N = a.shape
    nb = N // 512
    v = a.reshape(K // 2, 2, nb, 512).transpose(0, 2, 1, 3)
    return np.ascontiguousarray(v.reshape(K // 2, 2 * N))


def _pack_w(a, pad):
    """[K, N] -> [K/2, 2*pad], col = i*pad + n (planes padded to 16B mult)."""
    K, N = a.shape
    v = np.zeros((K // 2, 2, pad), dtype=a.dtype)
    v[:, :, :N] = a.reshape(K // 2, 2, N)
    return np.ascontiguousarray(v.reshape(K // 2, 2 * pad))


def kernel(x0, x1, W0, b0, W1, b1):
    x0 = np.asarray(x0, dtype=np.float32)
    x1 = np.asarray(x1, dtype=np.float32)
    W0 = np.asarray(W0, dtype=np.float32)
    W1 = np.asarray(W1, dtype=np.float32)
    b0 = np.asarray(b0, dtype=np.float32)
    b1 = np.asarray(b1, dtype=np.float32)
    B = x0.shape[0]
    assert B == 8, f"expected batch 8, got {B}"

    use_bias = bool(np.any(b0) or np.any(b1))
    nc = _get_program(use_bias)

    shared = _host_consts()
    whcols = np.array([86 * a + 2 + c for a in range(NA) for c in range(2)])
    for li, (W, bb) in enumerate(zip((W0, W1), (b0, b1))):
        wt = np.ascontiguousarray(W.T)  # [C, 1548]
        if use_bias:
            wt = np.concatenate(
                [wt, bb.reshape(1, -1), np.zeros_like(bb).reshape(1, -1)],
                axis=0)
        w8 = (wt * SC_W).astype(E4M3)
        shared[f"wt{li}"] = _pack_w(w8, 2048)
        if li == 1:
            # wh correction weights: v8 ~ w*16 (for the x-residual term),
            # q8 = W-quantization residual (256w - w8)
            v8 = (wt[:, whcols] * SC_X).astype(E4M3)
            q8 = (wt[:, whcols] * SC_W
                  - w8[:, whcols].astype(np.float32)).astype(E4M3)
            wres = np.concatenate(
                [v8.astype(np.float32), q8.astype(np.float32),
                 np.zeros((wt.shape[0], 8), np.float32)],
                axis=1).astype(E4M3)
            shared["wres"] = _pack_w(wres, 80)

    in_maps = []
    for i in range(B):
        m = dict(shared)
        for li, (x, lv) in enumerate(zip((x0, x1), LEVELS)):
            xs = x[i].reshape(lv["C"], lv["HW"]) * SC_X
            if use_bias:
                xs = np.concatenate(
                    [xs, np.full((1, lv["HW"]), SC_X, np.float32),
                     np.zeros((1, lv["HW"]), np.float32)], axis=0)
            x8 = xs.astype(E4M3)
            m[f"xs{li}"] = _pack_x(x8)
            if li == 1:
                r8 = ((xs - x8.astype(np.float32)) * SC_X).astype(E4M3)
                m["r8"] = _pack_x(r8)
        in_maps.append(m)

    res = bass_utils.run_bass_kernel_spmd(nc, in_maps, core_ids=list(range(B)))
    out = np.stack([res.results[i]["out"] for i in range(B)], axis=0)
    return np.ascontiguousarray(out.astype(np.float32))
